# revision 10
# baseline (speedup 1.0000x reference)
"""DeepseekV3 MLA flash-attention prefill kernel for 8 Trainium2 NeuronCores.

Sharding strategy (SPMD, one program for all 8 cores):
  Stage A (sequence-parallel, row layout): core c computes the low-rank
    down-projections q_a = rms_norm(X @ Wqa), c_kv = rms_norm(ckv[:, :512]),
    k_pe(roped) for its 256 rows, transposes them to feature-major with the
    PE array, and AllGathers them (kv first, so stage B can overlap).
  Stage B (head-parallel): core c owns heads {2c, 2c+1}: up-projections
    (Wqb / Wkvb), RoPE on q_pe, causal attention in (k, q) layout
    (softmax without max-subtraction; fully-masked k-blocks skipped,
    diagonal blocks mask-multiplied), per-q normalization folded into the
    attn^T eviction.  Heads are processed sequentially, each followed by
    its own AllToAll so the first exchange overlaps the second head.
  The AllToAlls convert head-sharded attn^T to sequence-sharded; each core
    then computes its 256 output rows against the full Wo. Host concatenates.
"""

import sys

if '/opt/trn_rl_repo' not in sys.path:
    sys.path.insert(0, '/opt/trn_rl_repo')

import numpy as np
import ml_dtypes

import concourse.bass as bass
import concourse.mybir as mybir
import concourse.tile as tile
from concourse import bacc
from concourse.bass_utils import run_bass_kernel_spmd

f32 = mybir.dt.float32
f32r = mybir.dt.float32r
bf16 = mybir.dt.bfloat16
i32 = mybir.dt.int32
AF = mybir.ActivationFunctionType
ALU = mybir.AluOpType

NC_ = 8            # cores
S = 2048           # sequence
HID = 2048
QLR = 1536         # q lora rank
KVLR = 512         # kv lora rank
ROPE = 64
NOPE = 128
VD = 128
NH = 16
HPC = NH // NC_    # heads per core = 2
SL = S // NC_      # rows per core = 256
PANEL = 512        # q panel width
NPANEL = S // PANEL
NKB = S // 128     # 16 k blocks
QCH = QLR // 128   # 12
KCH = KVLR // 128  # 4
HCH = HID // 128   # 16
THETA = 10000.0
SM_SCALE = float((NOPE + ROPE) ** -0.5)
PI = float(np.pi)

DT = f32r          # matmul dtype: f32r or bf16

_CACHE = {}


def _range_reduce_sin(nc, pool, src_ap, P, W, bias, name, tag):
    """sin(src + bias) with range reduction to [-pi, pi]. src may be PSUM."""
    t0 = pool.tile([P, W], f32, name=f"{name}_t0", tag="rr0", bufs=1)
    ti = pool.tile([P, W], i32, name=f"{name}_ti", tag="rr1", bufs=1)
    tf = pool.tile([P, W], f32, name=f"{name}_tf", tag="rr2", bufs=1)
    arg = pool.tile([P, W], f32, name=f"{name}_arg", tag="rr3", bufs=1)
    res = pool.tile([P, W], f32, name=f"{name}_sin", tag=tag, bufs=2)
    nc.vector.tensor_scalar(out=t0[:], in0=src_ap, scalar1=bias, scalar2=None, op0=ALU.add)
    nc.vector.tensor_scalar(out=tf[:], in0=t0[:], scalar1=1.0 / (2 * PI), scalar2=None, op0=ALU.mult)
    nc.vector.tensor_copy(ti[:], tf[:])
    nc.vector.tensor_copy(tf[:], ti[:])
    nc.vector.scalar_tensor_tensor(out=arg[:], in0=tf[:], scalar=-2 * PI, in1=t0[:], op0=ALU.mult, op1=ALU.add)
    nc.scalar.activation(res[:], arg[:], AF.Sin)
    return res


def build_program(dt):
    nc = bacc.Bacc("TRN2", target_bir_lowering=False, debug=False, num_devices=NC_)

    def din(name, shape):
        return nc.dram_tensor(name, shape, dt, kind="ExternalInput")

    # ---- external I/O (per-core data) ----
    x_t = din("x_t", [HID, SL])                 # X rows, transposed (hid-major)
    pos = nc.dram_tensor("pos", [1, SL], f32, kind="ExternalInput")
    pos_all = nc.dram_tensor("pos_all", [1, S], f32, kind="ExternalInput")
    wqa = din("wqa", [HID, QLR])
    wkva = din("wkva", [HID, KVLR + ROPE])
    wqb = din("wqb", [QLR, HPC * 256])          # [nope|pe_d|rot] per head
    wkvb_k = din("wkvb_k", [KVLR, HPC * NOPE])
    wkvb_v = din("wkvb_v", [KVLR, HPC * VD])
    wo = din("wo", [NH * VD, HID])
    cmask = din("cmask", [4 * 128, PANEL])
    ident = din("ident", [128, 128])
    ones_col = din("ones_col", [128, 1])
    ones_row = nc.dram_tensor("ones_row", [1, 128], f32, kind="ExternalInput")
    invf_row = nc.dram_tensor("invf_row", [1, ROPE], f32, kind="ExternalInput")
    invf_bc = nc.dram_tensor("invf_bc", [128, ROPE], f32, kind="ExternalInput")
    invf_col = nc.dram_tensor("invf_col", [ROPE, 1], f32, kind="ExternalInput")
    out_loc = nc.dram_tensor("out_loc", [SL, HID], f32, kind="ExternalOutput")

    NAG_KV = KCH + 1

    with tile.TileContext(nc) as tc:
        with tc.tile_pool(name="dram", bufs=1, space="DRAM") as dpool, \
             tc.tile_pool(name="consts", bufs=1) as cpool:
            ag_in_kv = dpool.tile([NAG_KV * 128, SL], dt)
            ag_out_kv = dpool.tile([NC_ * NAG_KV * 128, SL], dt, addr_space="Shared")
            ag_in_q = dpool.tile([QCH * 128, SL], dt)
            ag_out_q = dpool.tile([NC_ * QCH * 128, SL], dt, addr_space="Shared")
            a2a_in = [dpool.tile([NC_ * VD, SL], dt, name=f"a2a_in{h}") for h in range(HPC)]
            a2a_out = [dpool.tile([NC_ * VD, SL], dt, name=f"a2a_out{h}") for h in range(HPC)]

            ocol = cpool.tile([128, 1], dt)
            orow = cpool.tile([1, 128], f32r)
            id_t = cpool.tile([128, 128], dt)
            invr_t = cpool.tile([1, ROPE], f32r)
            invbc_t = cpool.tile([128, ROPE], f32)
            invc_t = cpool.tile([ROPE, 1], f32)
            pos_all_t = cpool.tile([1, S], f32r)
            pos_t = cpool.tile([1, SL], f32r)
            nc.sync.dma_start(out=ocol[:], in_=ones_col[:])
            nc.sync.dma_start(out=orow[:], in_=ones_row[:].bitcast(f32r))
            nc.sync.dma_start(out=id_t[:], in_=ident[:])
            nc.sync.dma_start(out=invr_t[:], in_=invf_row[:].bitcast(f32r))
            nc.sync.dma_start(out=invbc_t[:], in_=invf_bc[:])
            nc.sync.dma_start(out=invc_t[:], in_=invf_col[:])
            nc.sync.dma_start(out=pos_all_t[:], in_=pos_all[:].bitcast(f32r))
            nc.sync.dma_start(out=pos_t[:], in_=pos[:].bitcast(f32r))

            # ================= Stage A: seq-parallel down projections (row layout) =====
            with tc.tile_pool(name="sa_x", bufs=1) as xp, \
                 tc.tile_pool(name="sa_w", bufs=4) as wp, \
                 tc.tile_pool(name="sa_tmp", bufs=2) as tp, \
                 tc.tile_pool(name="sa_ps", bufs=1, space="PSUM") as pp, \
                 tc.tile_pool(name="sa_tps", bufs=2, space="PSUM") as tpp:

                xts = {}
                for hc in range(HCH):
                    for rb in range(SL // 128):
                        t = xp.tile([128, 128], dt, name=f"xt{hc}_{rb}")
                        nc.sync.dma_start(out=t[:], in_=x_t[128 * hc:128 * (hc + 1), 128 * rb:128 * (rb + 1)])
                        xts[(hc, rb)] = t

                def stage_a_block(rb, specs, norm_len, dst, dst_col):
                    """specs: (w_dram, col0, width, name) list. Row-block rb of
                    [X @ W][:, cols] -> rms-normalize over norm_len cols ->
                    transpose 128-col chunks -> DMA into dst at dst_col."""
                    psums = []
                    for (w_dram, c0, wd, nm) in specs:
                        ps = pp.tile([128, wd], f32, name=f"ps_{nm}_{rb}", tag=f"ps_{nm}", bufs=1)
                        for hc in range(HCH):
                            wt = wp.tile([128, wd], dt, name=f"w_{nm}_{rb}_{hc}", tag=f"w_{nm}", bufs=8)
                            nc.sync.dma_start(out=wt[:], in_=w_dram[128 * hc:128 * (hc + 1), c0:c0 + wd])
                            nc.tensor.matmul(ps[:], xts[(hc, rb)][:], wt[:], start=(hc == 0), stop=(hc == HCH - 1))
                        psums.append(ps)
                    accs = []
                    for i, ps in enumerate(psums):
                        sq = tp.tile([128, ps.shape[1]], f32, name=f"sq_{rb}_{i}", tag="sq", bufs=2)
                        acc = tp.tile([128, 1], f32, name=f"acc_{rb}_{i}", tag=f"acc{i}", bufs=2)
                        nc.scalar.activation(sq[:], ps[:], AF.Square, accum_out=acc[:])
                        accs.append(acc)
                    tot = tp.tile([128, 1], f32, name=f"tot_{rb}", tag="tot", bufs=2)
                    if len(accs) == 1:
                        nc.vector.tensor_scalar(out=tot[:], in0=accs[0][:], scalar1=1.0 / norm_len, scalar2=None, op0=ALU.mult)
                    else:
                        nc.vector.tensor_add(tot[:], accs[0][:], accs[1][:])
                        for a in accs[2:]:
                            nc.vector.tensor_add(tot[:], tot[:], a[:])
                        nc.vector.tensor_scalar(out=tot[:], in0=tot[:], scalar1=1.0 / norm_len, scalar2=None, op0=ALU.mult)
                    rms = tp.tile([128, 1], f32, name=f"rms_{rb}", tag="rms", bufs=2)
                    nc.scalar.activation(rms[:], tot[:], AF.Sqrt)
                    rsc = tp.tile([128, 1], f32, name=f"rsc_{rb}", tag="rsc", bufs=2)
                    nc.vector.reciprocal(rsc[:], rms[:])
                    coff = 0
                    for ps in psums:
                        wd = ps.shape[1]
                        sc = tp.tile([128, wd], dt, name=f"scl_{rb}_{coff}", tag="scl", bufs=2)
                        nc.vector.tensor_scalar(out=sc[:], in0=ps[:], scalar1=rsc[:], scalar2=None, op0=ALU.mult)
                        for sub in range(0, wd, 128):
                            sw = min(128, wd - sub)
                            tps = tpp.tile([sw, 128], dt, name=f"tps_{rb}_{coff}_{sub}", tag="tps", bufs=2)
                            nc.tensor.transpose(tps[:], sc[:, sub:sub + sw], id_t[:])
                            st = tp.tile([sw, 128], dt, name=f"st_{rb}_{coff}_{sub}", tag="st", bufs=3)
                            nc.vector.tensor_copy(st[:], tps[:])
                            nc.sync.dma_start(
                                out=dst[dst_col + coff + sub:dst_col + coff + sub + sw,
                                        128 * rb:128 * (rb + 1)],
                                in_=st[:])
                        coff += wd

                # kv + pe first (so AG1 can fire early)
                for rb in range(SL // 128):
                    stage_a_block(rb, [(wkva, 0, 512, "kv")], KVLR, ag_in_kv, 0)
                    # k_pe: no norm; rope in row layout, then transpose
                    ps = pp.tile([128, ROPE], f32, name=f"ps_pe_{rb}", tag="ps_pe", bufs=1)
                    for hc in range(HCH):
                        wt = wp.tile([128, ROPE], dt, name=f"w_pe_{rb}_{hc}", tag="w_pe", bufs=4)
                        nc.sync.dma_start(out=wt[:], in_=wkva[128 * hc:128 * (hc + 1), KVLR:KVLR + ROPE])
                        nc.tensor.matmul(ps[:], xts[(hc, rb)][:], wt[:], start=(hc == 0), stop=(hc == HCH - 1))
                    tb = tpp.tile([128, ROPE], f32, name=f"tb_{rb}", tag="tb", bufs=1)
                    nc.tensor.matmul(tb[:], pos_t[0:1, 128 * rb:128 * (rb + 1)], orow[0:1, 0:ROPE], start=True, stop=True)
                    emb = tp.tile([128, ROPE], f32, name=f"emb_{rb}", tag="emb", bufs=2)
                    nc.vector.tensor_mul(emb[:], tb[:], invbc_t[:])
                    sin_t = _range_reduce_sin(nc, tp, emb[:], 128, ROPE, 0.0, f"sa_s{rb}", "sin_s")
                    cos_t = _range_reduce_sin(nc, tp, emb[:], 128, ROPE, PI / 2, f"sa_c{rb}", "sin_c")
                    krot = tp.tile([128, ROPE], f32, name=f"krot_{rb}", tag="krot", bufs=2)
                    nc.vector.tensor_scalar(out=krot[:, 0:32], in0=ps[:, 32:64], scalar1=-1.0, scalar2=None, op0=ALU.mult)
                    nc.vector.tensor_copy(krot[:, 32:64], ps[:, 0:32])
                    kro = tp.tile([128, ROPE], f32, name=f"kro_{rb}", tag="kro", bufs=2)
                    nc.vector.tensor_mul(kro[:], ps[:], cos_t[:])
                    krs = tp.tile([128, ROPE], f32, name=f"krs_{rb}", tag="krs", bufs=2)
                    nc.vector.tensor_mul(krs[:], krot[:], sin_t[:])
                    kfin = tp.tile([128, ROPE], dt, name=f"kfin_{rb}", tag="kfin", bufs=2)
                    nc.vector.tensor_add(kfin[:], kro[:], krs[:])
                    tps = tpp.tile([ROPE, 128], dt, name=f"tpspe_{rb}", tag="tps", bufs=2)
                    nc.tensor.transpose(tps[:], kfin[:], id_t[:])
                    st = tp.tile([ROPE, 128], dt, name=f"stpe_{rb}", tag="st", bufs=3)
                    nc.vector.tensor_copy(st[:], tps[:])
                    nc.sync.dma_start(out=ag_in_kv[KCH * 128:KCH * 128 + ROPE, 128 * rb:128 * (rb + 1)], in_=st[:])

                nc.gpsimd.collective_compute(
                    "AllGather", ALU.bypass,
                    replica_groups=[list(range(NC_))],
                    ins=[ag_in_kv[:]], outs=[ag_out_kv[:]],
                )

                for rb in range(SL // 128):
                    stage_a_block(rb, [(wqa, 0, 512, "q0"), (wqa, 512, 512, "q1"), (wqa, 1024, 512, "q2")],
                                  QLR, ag_in_q, 0)

                nc.gpsimd.collective_compute(
                    "AllGather", ALU.bypass,
                    replica_groups=[list(range(NC_))],
                    ins=[ag_in_q[:]], outs=[ag_out_q[:]],
                )

            def agkv(r, o):
                base = (r * NAG_KV + o) * 128
                return ag_out_kv[base:base + 128, :]

            def agq(r, o):
                base = (r * QCH + o) * 128
                return ag_out_q[base:base + 128, :]

            # ================= Stage B: head-parallel attention =================
            wop_cm = tc.tile_pool(name="wo_w", bufs=1)
            wop = wop_cm.__enter__()
            wo_tiles = []
            for c in range(HCH):
                for col in range(HID // 512):
                    t = wop.tile([128, 512], dt, name=f"wo_c{c}_{col}")
                    nc.sync.dma_start(out=t[:], in_=wo[128 * c:128 * (c + 1), 512 * col:512 * (col + 1)])
                    wo_tiles.append((c, col, t))
            wo_map = {(c, col): t for (c, col, t) in wo_tiles}
            with tc.tile_pool(name="sb_res", bufs=1) as rp, \
                 tc.tile_pool(name="sb_qa", bufs=1) as qap, \
                 tc.tile_pool(name="sb_tmp", bufs=2) as tp, \
                 tc.tile_pool(name="sb_pt", bufs=3) as ptp, \
                 tc.tile_pool(name="sb_ps", bufs=2, space="PSUM") as pp, \
                 tc.tile_pool(name="sb_ps1", bufs=1, space="PSUM") as pp1:

                kpe_g = rp.tile([ROPE, S], dt, name="kpe_g")
                for r in range(NC_):
                    nc.sync.dma_start(out=kpe_g[:, SL * r:SL * (r + 1)], in_=agkv(r, KCH)[0:ROPE, :])

                wqb_t = []
                for l in range(QCH):
                    t = rp.tile([128, HPC * 256], dt, name=f"wqb_t{l}")
                    nc.sync.dma_start(out=t[:], in_=wqb[128 * l:128 * (l + 1), :])
                    wqb_t.append(t)
                wkk_t = []
                wkv_t = []
                for l in range(KCH):
                    t = rp.tile([128, HPC * NOPE], dt, name=f"wkk_t{l}")
                    nc.sync.dma_start(out=t[:], in_=wkvb_k[128 * l:128 * (l + 1), :])
                    wkk_t.append(t)
                    t2 = rp.tile([128, HPC * VD], dt, name=f"wkv_t{l}")
                    nc.sync.dma_start(out=t2[:], in_=wkvb_v[128 * l:128 * (l + 1), :])
                    wkv_t.append(t2)
                cm_t = []
                for j in range(4):
                    t = rp.tile([128, PANEL], dt, name=f"cm_t{j}")
                    nc.sync.dma_start(out=t[:], in_=cmask[128 * j:128 * (j + 1), :])
                    cm_t.append(t)

                # K^T and V (both heads); ckv_g freed afterwards
                kT = [rp.tile([128, S], dt, name=f"kT{h}") for h in range(HPC)]
                v_t = [rp.tile([128, HPC * VD], dt, name=f"v_t{kb}") for kb in range(NKB)]
                with tc.tile_pool(name="sb_ckv", bufs=1) as ckvp:
                    ckv_g = []
                    for j in range(KCH):
                        t = ckvp.tile([128, S], dt, name=f"ckv_g{j}")
                        for r in range(NC_):
                            nc.sync.dma_start(out=t[:, SL * r:SL * (r + 1)], in_=agkv(r, j))
                        ckv_g.append(t)
                    for h in range(HPC):
                        for kc in range(S // 512):
                            ps = pp.tile([128, 512], f32, name=f"kt_ps{h}_{kc}", tag="mm_ps", bufs=2)
                            for l in range(KCH):
                                nc.tensor.matmul(ps[:], wkk_t[l][:, NOPE * h:NOPE * (h + 1)],
                                                 ckv_g[l][:, 512 * kc:512 * (kc + 1)],
                                                 start=(l == 0), stop=(l == KCH - 1))
                            nc.vector.tensor_copy(kT[h][:, 512 * kc:512 * (kc + 1)], ps[:])
                    for kb in range(NKB):
                        ps = pp.tile([128, HPC * VD], f32, name=f"v_ps{kb}", tag="mm_ps", bufs=2)
                        for l in range(KCH):
                            nc.tensor.matmul(ps[:], ckv_g[l][:, 128 * kb:128 * (kb + 1)], wkv_t[l][:],
                                             start=(l == 0), stop=(l == KCH - 1))
                        nc.vector.tensor_copy(v_t[kb][:], ps[:])

                for h in range(HPC):
                    hcol = 256 * h
                    for p in range(NPANEL):
                        qs = slice(PANEL * p, PANEL * (p + 1))
                        qa_p = []
                        for l in range(QCH):
                            t = qap.tile([128, PANEL], dt, name=f"qa_p{h}_{p}_{l}", tag=f"qa_p{l}", bufs=1)
                            for r in range(2):
                                nc.sync.dma_start(out=t[:, SL * r:SL * (r + 1)], in_=agq(2 * p + r, l))
                            qa_p.append(t)

                        tb = pp1.tile([ROPE, PANEL], f32, name=f"tbp{h}_{p}", tag="bc_ps", bufs=1)
                        nc.tensor.matmul(tb[:], orow[0:1, 0:ROPE], pos_all_t[:, qs], start=True, stop=True)
                        embp = tp.tile([ROPE, PANEL], f32, name=f"embp{h}_{p}", tag="embp", bufs=2)
                        nc.vector.tensor_scalar(out=embp[:], in0=tb[:], scalar1=invc_t[:], scalar2=None, op0=ALU.mult)
                        sin_p = _range_reduce_sin(nc, tp, embp[:], ROPE, PANEL, 0.0, f"sb_s{h}_{p}", "sin_s")
                        cos_p = _range_reduce_sin(nc, tp, embp[:], ROPE, PANEL, PI / 2, f"sb_c{h}_{p}", "sin_c")

                        nkb = 4 * (p + 1)
                        ps_qn = pp.tile([128, PANEL], f32, name=f"qn_ps{h}_{p}", tag="mm_ps", bufs=2)
                        for l in range(QCH):
                            nc.tensor.matmul(ps_qn[:], wqb_t[l][:, hcol:hcol + NOPE], qa_p[l][:],
                                             start=(l == 0), stop=(l == QCH - 1))
                        ps_qr = pp.tile([128, PANEL], f32, name=f"qr_ps{h}_{p}", tag="mm_ps", bufs=2)
                        for l in range(QCH):
                            nc.tensor.matmul(ps_qr[:], wqb_t[l][:, hcol + NOPE:hcol + 256], qa_p[l][:],
                                             start=(l == 0), stop=(l == QCH - 1))
                        qn_t = tp.tile([128, PANEL], dt, name=f"qn_t{h}_{p}", tag="qn_t", bufs=2)
                        nc.vector.tensor_copy(qn_t[:], ps_qn[:])
                        qt1 = tp.tile([ROPE, PANEL], f32, name=f"qt1_{h}_{p}", tag="qt1", bufs=2)
                        nc.vector.tensor_mul(qt1[:], ps_qr[0:ROPE, :], cos_p[:])
                        qt2 = tp.tile([ROPE, PANEL], f32, name=f"qt2_{h}_{p}", tag="qt2", bufs=2)
                        nc.vector.tensor_mul(qt2[:], ps_qr[ROPE:2 * ROPE, :], sin_p[:])
                        qp_t = tp.tile([ROPE, PANEL], dt, name=f"qp_t{h}_{p}", tag="qp_t", bufs=2)
                        nc.vector.tensor_add(qp_t[:], qt1[:], qt2[:])

                        ps_at = pp.tile([128, PANEL], f32, name=f"at_ps{h}_{p}", tag="at_ps", bufs=1)
                        ps_sum = pp1.tile([1, PANEL], f32, name=f"sum_ps{h}_{p}", tag="sum_ps", bufs=1)
                        for kb in range(nkb):
                            ps_sc = pp.tile([128, PANEL], f32, name=f"sc_ps{h}_{p}_{kb}", tag="sc_ps", bufs=3)
                            nc.tensor.matmul(ps_sc[:], kT[h][:, 128 * kb:128 * (kb + 1)], qn_t[:],
                                             start=True, stop=False)
                            nc.tensor.matmul(ps_sc[:], kpe_g[:, 128 * kb:128 * (kb + 1)], qp_t[:],
                                             start=False, stop=True)
                            pt = ptp.tile([128, PANEL], dt, name=f"pt{h}_{p}_{kb}", tag="pt", bufs=3)
                            nc.scalar.activation(pt[:], ps_sc[:], AF.Exp, scale=SM_SCALE)
                            if kb >= 4 * p:
                                j = kb - 4 * p
                                nc.gpsimd.affine_select(
                                    out=pt[:], in_=pt[:],
                                    pattern=[[1, PANEL]],
                                    compare_op=ALU.is_ge,
                                    fill=0.0,
                                    base=-128 * j,
                                    channel_multiplier=-1)
                            nc.tensor.matmul(ps_sum[:], ocol[:], pt[:],
                                             start=(kb == 0), stop=(kb == nkb - 1))
                            nc.tensor.matmul(ps_at[:], v_t[kb][:, VD * h:VD * (h + 1)], pt[:],
                                             start=(kb == 0), stop=(kb == nkb - 1))
                        rec = tp.tile([1, PANEL], f32r, name=f"rec{h}_{p}", tag="rec", bufs=2)
                        with nc.allow_low_precision(reason="f32r rounding of softmax recip"):
                            nc.vector.reciprocal(rec[:], ps_sum[:])
                        bc = pp1.tile([128, PANEL], f32, name=f"bc_ps{h}_{p}", tag="bc_ps", bufs=1)
                        nc.tensor.matmul(bc[:], orow[:], rec[:], start=True, stop=True)
                        bc_sb = tp.tile([128, PANEL], f32, name=f"bc_sb{h}_{p}", tag="bc_sb", bufs=2)
                        nc.vector.tensor_copy(bc_sb[:], bc[:])
                        at_p = tp.tile([128, PANEL], dt, name=f"at_p{h}_{p}", tag="at_p", bufs=2)
                        nc.vector.tensor_mul(at_p[:], ps_at[:], bc_sb[:])
                        for r in range(2):
                            j = 2 * p + r
                            nc.sync.dma_start(
                                out=a2a_in[h][j * VD:(j + 1) * VD, :],
                                in_=at_p[:, SL * r:SL * (r + 1)])
                    nc.gpsimd.collective_compute(
                        "AllToAll", ALU.bypass,
                        replica_groups=[list(range(NC_))],
                        ins=[a2a_in[h][:]], outs=[a2a_out[h][:]],
                    )

            # ================= Wo stage: seq-parallel output projection =================
            with tc.tile_pool(name="wo_res", bufs=1) as rp, \
                 tc.tile_pool(name="wo_tmp", bufs=3) as tp, \
                 tc.tile_pool(name="wo_ps", bufs=2, space="PSUM") as pp:
                att_g = []
                for c in range(HCH):
                    j, h = divmod(c, HPC)
                    t = rp.tile([128, SL], dt, name=f"att_g{c}")
                    nc.sync.dma_start(out=t[:], in_=a2a_out[h][128 * j:128 * (j + 1), :])
                    att_g.append(t)
                for col in range(HID // 512):
                    for qb in range(SL // 128):
                        ps = pp.tile([128, 512], f32, name=f"o_ps{col}_{qb}", tag="o_ps", bufs=2)
                        for c in range(HCH):
                            nc.tensor.matmul(ps[:], att_g[c][:, 128 * qb:128 * (qb + 1)], wo_map[(c, col)][:],
                                             start=(c == 0), stop=(c == HCH - 1))
                        osb = tp.tile([128, 512], f32, name=f"osb{col}_{qb}", tag="osb", bufs=3)
                        nc.vector.tensor_copy(osb[:], ps[:])
                        nc.sync.dma_start(out=out_loc[128 * qb:128 * (qb + 1), 512 * col:512 * (col + 1)], in_=osb[:])
            wop_cm.__exit__(None, None, None)

    nc.compile()
    return nc


def _to_dt(a, dt):
    if dt == bf16:
        return np.ascontiguousarray(a.astype(ml_dtypes.bfloat16))
    return np.ascontiguousarray(a.astype(np.float32))


def _prepare_inputs(dt, hidden_states, position_ids, Wqa, qa_ln_w, Wqb, Wkva, kv_ln_w, Wkvb, Wo):
    perm = np.concatenate([np.arange(0, ROPE, 2), np.arange(1, ROPE, 2)])
    X = np.asarray(hidden_states, np.float32).reshape(S, HID)
    pos_f = np.ascontiguousarray(np.asarray(position_ids, np.float32).reshape(1, S))
    Wqa = np.asarray(Wqa, np.float32)
    Wkva = np.asarray(Wkva, np.float32)
    Wkva_p = Wkva.copy()
    Wkva_p[:, KVLR:] = Wkva[:, KVLR:][:, perm]
    wqb_base = np.asarray(Wqb, np.float32) * np.asarray(qa_ln_w, np.float32)[:, None]
    wkvb_base = np.asarray(Wkvb, np.float32) * np.asarray(kv_ln_w, np.float32)[:, None]
    Wo = np.asarray(Wo, np.float32)

    head_blocks = []
    for h in range(NH):
        cols = wqb_base[:, 192 * h:192 * (h + 1)]
        nope = cols[:, :NOPE]
        pe_d = cols[:, NOPE:][:, perm]
        rot = np.concatenate([-pe_d[:, 32:], pe_d[:, :32]], axis=1)
        head_blocks.append(np.concatenate([nope, pe_d, rot], axis=1))  # (1536, 256)
    k_blocks = [wkvb_base[:, 256 * h:256 * h + NOPE] for h in range(NH)]
    v_blocks = [wkvb_base[:, 256 * h + NOPE:256 * (h + 1)] for h in range(NH)]

    cmask = np.zeros((4, 128, PANEL), np.float32)
    for j in range(4):
        base = 128 * j
        for q in range(PANEL):
            k_lim = q - base
            if k_lim >= 0:
                cmask[j, :min(k_lim + 1, 128), q] = 1.0
    cmask = cmask.reshape(4 * 128, PANEL)
    inv = (1.0 / (THETA ** (np.arange(0, ROPE, 2, dtype=np.float32) / ROPE))).astype(np.float32)
    invf_np = np.concatenate([inv, inv])

    wqa_d = _to_dt(Wqa, dt)
    wkva_d = _to_dt(Wkva_p, dt)
    wo_d = _to_dt(Wo, dt)
    cmask_d = _to_dt(cmask, dt)
    ident_d = _to_dt(np.eye(128, dtype=np.float32), dt)
    ones_col_d = _to_dt(np.ones((128, 1), np.float32), dt)

    in_maps = []
    for c in range(NC_):
        rows = slice(SL * c, SL * (c + 1))
        in_maps.append({
            "x_t": _to_dt(X[rows, :].T, dt),
            "pos": np.ascontiguousarray(pos_f[:, rows]),
            "pos_all": pos_f,
            "wqa": wqa_d,
            "wkva": wkva_d,
            "wqb": _to_dt(np.concatenate([head_blocks[HPC * c + h] for h in range(HPC)], axis=1), dt),
            "wkvb_k": _to_dt(np.concatenate([k_blocks[HPC * c + h] for h in range(HPC)], axis=1), dt),
            "wkvb_v": _to_dt(np.concatenate([v_blocks[HPC * c + h] for h in range(HPC)], axis=1), dt),
            "wo": wo_d,
            "cmask": cmask_d,
            "ident": ident_d,
            "ones_col": ones_col_d,
            "ones_row": np.ones((1, 128), np.float32),
            "invf_row": invf_np.reshape(1, ROPE).copy(),
            "invf_bc": np.broadcast_to(invf_np.reshape(1, ROPE), (128, ROPE)).copy(),
            "invf_col": invf_np.reshape(ROPE, 1).copy(),
        })
    return in_maps


def run(inputs, trace=False, trace_cores=None, dt=None):
    dt = dt if dt is not None else DT
    key = ("nc", str(dt))
    if key not in _CACHE:
        _CACHE[key] = build_program(dt)
    nc = _CACHE[key]
    in_maps = _prepare_inputs(dt, **inputs)
    res = run_bass_kernel_spmd(nc, in_maps, list(range(NC_)), trace=trace,
                               trace_cores=trace_cores)
    out = np.concatenate([res.results[c]["out_loc"] for c in range(NC_)], axis=0)
    return out.reshape(1, S, HID), res


def kernel(**inputs) -> np.ndarray:
    out, _ = run(inputs, trace=False)
    return out
